# Initial kernel scaffold
#
"""MLA-style attention (decoupled-RoPE) Trainium2 Bass kernel, 8-core SPMD.

Sharding: batch (2) x head-group (4 groups of 4 heads) = 8 cores.
Each core computes its batch's tokens for its 4 heads end-to-end
(projections -> rope -> causal softmax attention -> w_o partial),
returning a partial feature-major [D_MODEL, T] output; the host sums the
4 head-group partials per batch element and transposes.

All matmuls run as float32r (tf32-like) on the PE array. Scores are
computed k-major (S^T tiles [tok_k, tok_q]) so softmax denominators come
from a ones-vector matmul and P@V needs no transposes.
"""
import os
import sys

sys.path.insert(0, "/opt/trn_rl_repo")
os.environ.setdefault("JAX_PLATFORMS", "axon")

import numpy as np

import concourse.bacc as bacc
import concourse.mybir as mybir
import concourse.tile as tile
from concourse import bass_utils

# Model constants (hardcoded from the problem spec).
B, T, DM = 2, 2048, 2048
NH, DH, DL, DR = 16, 128, 512, 64
HPC = 4                      # heads per core
GF = HPC * DH                # 512 head-features per core
QRF = HPC * DR               # 256 rope features per core
SCALE = 1.0 / np.sqrt(DH + DR)
ROPE_BASE = 10000.0
N_CORES = 8

F32 = mybir.dt.float32
F32R = mybir.dt.float32r
EXP = mybir.ActivationFunctionType.Exp

TBS = 512                    # token block size in phase A
NTB = T // TBS               # 4
NKC = DM // 128              # 16 contraction chunks over d_model
NLC = DL // 128              # 4 contraction chunks over d_latent
NTC = T // 128               # 16 token chunks


def build_nc(reps=1):
    nc = bacc.Bacc("TRN2", target_bir_lowering=False, debug=False)

    # External inputs (per-core shards, host-prepared)
    xT = nc.dram_tensor("xT", [DM, T], F32R, kind="ExternalInput").ap()
    wq = nc.dram_tensor("wq", [DM, GF], F32R, kind="ExternalInput").ap()
    wdkv = nc.dram_tensor("wdkv", [DM, DL], F32R, kind="ExternalInput").ap()
    wk = nc.dram_tensor("wk", [DL, GF], F32R, kind="ExternalInput").ap()
    wv = nc.dram_tensor("wv", [DL, GF], F32R, kind="ExternalInput").ap()
    wqr = nc.dram_tensor("wqr", [DM, QRF], F32R, kind="ExternalInput").ap()
    wkr = nc.dram_tensor("wkr", [DM, DR], F32R, kind="ExternalInput").ap()
    wo = nc.dram_tensor("wo", [GF, DM], F32R, kind="ExternalInput").ap()
    cos2 = nc.dram_tensor("cos2", [128, T], F32, kind="ExternalInput").ap()
    ssin2 = nc.dram_tensor("ssin2", [128, T], F32, kind="ExternalInput").ap()
    dmask = nc.dram_tensor("dmask", [128, 2048], F32, kind="ExternalInput").ap()
    onesd = nc.dram_tensor("onesd", [128, 128], F32R, kind="ExternalInput").ap()

    outPT = nc.dram_tensor("outPT", [DM, T], F32, kind="ExternalOutput").ap()

    with tile.TileContext(nc) as tc, \
         nc.allow_low_precision(reason="float32r matmul operands are intentional"):
        with tc.tile_pool(name="dstage", bufs=1, space="DRAM") as dram, \
             tc.tile_pool(name="gfix", bufs=1) as gfix:
            # per-token-block staging tiles -> fine-grained cross-phase deps
            qTd = [dram.tile([GF, TBS], F32R, name=f"qTd{i}") for i in range(NTB)]
            qrTd = [dram.tile([QRF, TBS], F32R, name=f"qrTd{i}") for i in range(NTB)]
            oTd = [dram.tile([GF, 512], F32R, name=f"oTd{i}") for i in range(NTB)]

            mask_sb = gfix.tile([128, 2048], F32, name="mask_sb")
            ones_sb = gfix.tile([128, 1], F32R, name="ones_sb")
            ones1_sb = gfix.tile([1, 128], F32R, name="ones1_sb")

            from contextlib import ExitStack
            for _rep in range(reps):
                # k/v/k_rope stay resident in SBUF from projection to attention
                with tc.tile_pool(name="kv", bufs=1) as kvp:
                    kcs = [kvp.tile([128, T], F32R, name=f"kc_sb{h}")
                           for h in range(HPC)]
                    vhs = [kvp.tile([128, T], F32R, name=f"vh_sb{h}")
                           for h in range(HPC)]
                    krope_sb = kvp.tile([64, T], F32R, name="krope_sb")

                    bq_stack = ExitStack()
                    bq = bq_stack.enter_context(tc.tile_pool(name="bq", bufs=2))
                    # A2's weights: bulk SWDGE prefetch that overlaps A1
                    aw2_stack = ExitStack()
                    aw2 = aw2_stack.enter_context(tc.tile_pool(name="aw2", bufs=1))
                    wq_sb = aw2.tile([128, NKC * GF], F32R, name="wq_sb")
                    wqr_sb = aw2.tile([128, NKC * QRF], F32R, name="wqr_sb")
                    wkr_sb = aw2.tile([128, NKC * DR], F32R, name="wkr_sb")

                    def emit_prefetch():
                        # A2 weights ride SWDGE mid-A1 (off the critical
                        # start-up stream, done well before A2 needs them)
                        nc.gpsimd.dma_start(
                            out=wq_sb.rearrange("p (c f) -> p c f", c=NKC),
                            in_=wq.rearrange("(c p) f -> p c f", p=128))
                        nc.gpsimd.dma_start(
                            out=wqr_sb.rearrange("p (c f) -> p c f", c=NKC),
                            in_=wqr.rearrange("(c p) f -> p c f", p=128))
                        nc.gpsimd.dma_start(
                            out=wkr_sb.rearrange("p (c f) -> p c f", c=NKC),
                            in_=wkr.rearrange("(c p) f -> p c f", p=128))

                    _phase_a1(nc, tc, xT, wdkv, wk, wv, kcs, vhs, emit_prefetch)
                    # attention constants: loaded while A2 runs
                    nc.gpsimd.dma_start(out=mask_sb[:], in_=dmask)
                    nc.gpsimd.dma_start(out=ones_sb[:], in_=onesd[:, 0:1])
                    nc.gpsimd.dma_start(out=ones1_sb[:], in_=onesd[0:1, :])
                    _phase_a2(nc, tc, xT, wq_sb, wqr_sb, wkr_sb, cos2, ssin2,
                              qTd, qrTd, krope_sb)
                    aw2_stack.close()
                    with (
                        tc.tile_pool(name="cw", bufs=1) as cw,
                        tc.tile_pool(name="cps", bufs=2, space="PSUM") as cps,
                        tc.tile_pool(name="cot", bufs=8) as cot,
                        tc.tile_pool(name="cout", bufs=6) as cout,
                    ):
                        wo_sb = cw.tile([128, NLC * DM], F32R, name="wo_sb")

                        def emit_wo_chunk(c):
                            nc.gpsimd.dma_start(
                                out=wo_sb[:, c * DM:(c + 1) * DM],
                                in_=wo[c * 128:(c + 1) * 128, :])

                        _phase_b(nc, tc, mask_sb, ones_sb, ones1_sb, emit_wo_chunk, bq,
                                 qTd, qrTd, kcs, vhs, krope_sb, oTd)
                        _phase_c(nc, tc, wo_sb, oTd, outPT, cps, cot, cout)
                    bq_stack.close()

    nc.compile()
    return nc


def _phase_a1(nc, tc, xT, wdkv, wk, wv, kcs, vhs, emit_prefetch):
    """dkv projection -> k (feature-major) and v (token-major), written
    directly into the resident SBUF tiles used by attention."""
    with (
        tc.tile_pool(name="aw1", bufs=1) as aw,
        tc.tile_pool(name="ax1", bufs=4) as ax,
        tc.tile_pool(name="adkv", bufs=1) as adkv,
        tc.tile_pool(name="aps1", bufs=8, space="PSUM") as aps,
    ):
        wdkv_sb = aw.tile([128, NKC * DL], F32R, name="wdkv_sb")
        xts0 = []
        for c in range(NKC):
            nc.sync.dma_start(out=wdkv_sb[:, c * DL:(c + 1) * DL],
                              in_=wdkv[c * 128:(c + 1) * 128, :])
            xt = ax.tile([128, TBS], F32R, name="xt", tag="xt")
            nc.sync.dma_start(out=xt[:], in_=xT[c * 128:(c + 1) * 128, 0:TBS])
            xts0.append(xt)
        wk_sb = aw.tile([128, NLC * GF], F32R, name="wk_sb")
        wv_sb = aw.tile([128, NLC * GF], F32R, name="wv_sb")
        for c in range(NLC):
            nc.sync.dma_start(out=wk_sb[:, c * GF:(c + 1) * GF],
                              in_=wk[c * 128:(c + 1) * 128, :])
            nc.sync.dma_start(out=wv_sb[:, c * GF:(c + 1) * GF],
                              in_=wv[c * 128:(c + 1) * 128, :])

        for tb in range(NTB):
            if tb == 1:
                emit_prefetch()
            tsl = slice(tb * TBS, (tb + 1) * TBS)
            dkv_ps = [aps.tile([128, TBS], F32, name="ps", tag="ps") for _ in range(4)]
            for ki in range(NKC):
                if tb == 0:
                    xt = xts0[ki]
                else:
                    xt = ax.tile([128, TBS], F32R, name="xt", tag="xt")
                    nc.sync.dma_start(out=xt[:], in_=xT[ki * 128:(ki + 1) * 128, tsl])
                for fi in range(4):
                    nc.tensor.matmul(
                        dkv_ps[fi][:],
                        wdkv_sb[:, ki * DL + fi * 128: ki * DL + (fi + 1) * 128],
                        xt[:],
                        start=(ki == 0), stop=(ki == NKC - 1))
            dkv_sb = adkv.tile([128, NLC * TBS], F32R, name="dkv_sb", tag="dkv")
            for fi in range(4):
                dst = dkv_sb[:, fi * TBS:(fi + 1) * TBS]
                if fi % 2 == 0:
                    nc.vector.tensor_copy(dst, dkv_ps[fi][:])
                else:
                    nc.scalar.copy(dst, dkv_ps[fi][:])

            for fi in range(4):   # k feature tiles (one head each)
                kps = aps.tile([128, TBS], F32, name="ps", tag="ps")
                for c in range(NLC):
                    nc.tensor.matmul(
                        kps[:],
                        wk_sb[:, c * GF + fi * 128: c * GF + (fi + 1) * 128],
                        dkv_sb[:, c * TBS:(c + 1) * TBS],
                        start=(c == 0), stop=(c == NLC - 1))
                nc.scalar.copy(kcs[fi][:, tsl], kps[:])
            for ts in range(4):   # v token tiles (token-major)
                vps = aps.tile([128, TBS], F32, name="ps", tag="ps")
                for c in range(NLC):
                    nc.tensor.matmul(
                        vps[:],
                        dkv_sb[:, c * TBS + ts * 128: c * TBS + (ts + 1) * 128],
                        wv_sb[:, c * GF:(c + 1) * GF],
                        start=(c == 0), stop=(c == NLC - 1))
                cc = tb * 4 + ts
                for h in range(HPC):
                    nc.vector.tensor_copy(vhs[h][:, cc * 128:(cc + 1) * 128],
                                          vps[:, h * 128:(h + 1) * 128])


def _phase_a2(nc, tc, xT, wq_sb, wqr_sb, wkr_sb, cos2, ssin2,
              qTd, qrTd, krope_sb):
    """q / q_rot / k_rot projections + rope; q staged to DRAM per block,
    k_rope written into its resident SBUF tile."""
    with (
        tc.tile_pool(name="ax2", bufs=6) as ax,
        tc.tile_pool(name="astage", bufs=4) as ast,
        tc.tile_pool(name="arope", bufs=2) as arp,
        tc.tile_pool(name="aps2", bufs=8, space="PSUM") as aps,
    ):
        xts0 = []
        for c in range(NKC):
            xt = ax.tile([128, TBS], F32R, name="xt2", tag="xt2")
            nc.sync.dma_start(out=xt[:], in_=xT[c * 128:(c + 1) * 128, 0:TBS])
            xts0.append(xt)

        for tb in range(NTB):
            tsl = slice(tb * TBS, (tb + 1) * TBS)
            q_ps = [aps.tile([128, TBS], F32, name="ps2", tag="ps2") for _ in range(4)]
            qr_ps = [aps.tile([128, TBS], F32, name="ps2", tag="ps2") for _ in range(2)]
            kr_ps = aps.tile([64, TBS], F32, name="ps2", tag="ps2")
            cos_sl = arp.tile([128, TBS], F32, name="cos_sl", tag="cos_sl")
            nc.sync.dma_start(out=cos_sl[:], in_=cos2[:, tsl])
            ssin_sl = arp.tile([128, TBS], F32, name="ssin_sl", tag="ssin_sl")
            nc.sync.dma_start(out=ssin_sl[:], in_=ssin2[:, tsl])
            for ki in range(NKC):
                if tb == 0:
                    xt = xts0[ki]
                else:
                    xt = ax.tile([128, TBS], F32R, name="xt2", tag="xt2")
                    nc.sync.dma_start(out=xt[:], in_=xT[ki * 128:(ki + 1) * 128, tsl])
                for fi in range(4):
                    nc.tensor.matmul(
                        q_ps[fi][:],
                        wq_sb[:, ki * GF + fi * 128: ki * GF + (fi + 1) * 128],
                        xt[:], start=(ki == 0), stop=(ki == NKC - 1))
                for fi in range(2):
                    nc.tensor.matmul(
                        qr_ps[fi][:],
                        wqr_sb[:, ki * QRF + fi * 128: ki * QRF + (fi + 1) * 128],
                        xt[:], start=(ki == 0), stop=(ki == NKC - 1))
                nc.tensor.matmul(
                    kr_ps[:], wkr_sb[:, ki * DR:(ki + 1) * DR], xt[:],
                    start=(ki == 0), stop=(ki == NKC - 1))
            for fi in range(4):
                qst = ast.tile([128, TBS], F32R, name="st", tag="st")
                nc.vector.tensor_copy(qst[:], q_ps[fi][:])
                nc.scalar.dma_start(out=qTd[tb][fi * 128:(fi + 1) * 128, :], in_=qst[:])

            # rope on q_rot tiles (2 tiles of 2 heads) and k_rot
            for fi in range(2):
                raw = arp.tile([128, TBS], F32, name="rraw", tag="rraw")
                nc.scalar.copy(raw[:], qr_ps[fi][:])
                sh = arp.tile([128, TBS], F32, name="rsh", tag="rsh")
                for bb in range(4):
                    sb_ = bb ^ 1  # swap 32-halves within each 64-block
                    nc.gpsimd.dma_start(out=sh[bb * 32:(bb + 1) * 32, :],
                                        in_=raw[sb_ * 32:(sb_ + 1) * 32, :])
                m1 = arp.tile([128, TBS], F32, name="rm1", tag="rm1")
                nc.vector.tensor_mul(m1[:], raw[:], cos_sl[:])
                m2 = arp.tile([128, TBS], F32, name="rm2", tag="rm2")
                nc.vector.tensor_mul(m2[:], sh[:], ssin_sl[:])
                ro = ast.tile([128, TBS], F32R, name="st", tag="st")
                nc.vector.tensor_add(ro[:], m1[:], m2[:])
                nc.scalar.dma_start(out=qrTd[tb][fi * 128:(fi + 1) * 128, :], in_=ro[:])

            kraw = arp.tile([64, TBS], F32, name="rraw", tag="rraw")
            nc.scalar.copy(kraw[:], kr_ps[:])
            ksh = arp.tile([64, TBS], F32, name="rsh", tag="rsh")
            nc.gpsimd.dma_start(out=ksh[0:32, :], in_=kraw[32:64, :])
            nc.gpsimd.dma_start(out=ksh[32:64, :], in_=kraw[0:32, :])
            km1 = arp.tile([64, TBS], F32, name="rm1", tag="rm1")
            nc.vector.tensor_mul(km1[:], kraw[:], cos_sl[0:64, :])
            km2 = arp.tile([64, TBS], F32, name="rm2", tag="rm2")
            nc.vector.tensor_mul(km2[:], ksh[:], ssin_sl[0:64, :])
            nc.vector.tensor_add(krope_sb[:, tsl], km1[:], km2[:])


def _phase_b(nc, tc, mask_sb, ones_sb, ones1_sb, emit_wo_chunk, bq, qTd, qrTd, kcs, vhs,
             krope_sb, oTd):
    """Causal attention: q-block outer, head inner (so oTd completes
    front-to-back and phase C can overlap). k/v/k_rope already resident."""
    QBS = 512                 # q block size
    NQB = T // QBS            # 4
    with (
        tc.tile_pool(name="bpt", bufs=3) as bpt,
        tc.tile_pool(name="bout", bufs=3) as bout,
        tc.tile_pool(name="psst", bufs=3, space="PSUM") as psst,
        tc.tile_pool(name="pso", bufs=2, space="PSUM") as pso,
        tc.tile_pool(name="psdn", bufs=1, space="PSUM") as psdn,
    ):
        for qj in range(NQB):
            emit_wo_chunk(qj)
            qsl = slice(qj * QBS, (qj + 1) * QBS)
            nk = 4 * (qj + 1)         # causal: tok_k tiles 0..nk-1
            for h in range(HPC):
                kc_sb, vh_sb = kcs[h], vhs[h]
                qc = bq.tile([128, QBS], F32R, name="qc", tag="qc")
                nc.sync.dma_start(out=qc[:], in_=qTd[qj][h * 128:(h + 1) * 128, :])
                qr = bq.tile([64, QBS], F32R, name="qr", tag="qr")
                nc.sync.dma_start(out=qr[:], in_=qrTd[qj][h * 64:(h + 1) * 64, :])

                dn_ps = psdn.tile([1, QBS], F32, name="dn", tag="dn")
                o_ps = pso.tile([128, QBS], F32, name="o", tag="o")

                def emit_st(ki):
                    st = psst.tile([128, QBS], F32, name="stp", tag="stp")
                    nc.tensor.matmul(
                        st[:], kc_sb[:, ki * 128:(ki + 1) * 128], qc[:],
                        start=True, stop=False)
                    nc.tensor.matmul(
                        st[:], krope_sb[:, ki * 128:(ki + 1) * 128], qr[:],
                        start=False, stop=True)
                    return st

                sts = {0: emit_st(0), 1: emit_st(1)}
                for ki in range(nk):
                    if ki + 2 < nk:
                        sts[ki + 2] = emit_st(ki + 2)
                    st = sts.pop(ki)
                    pt = bpt.tile([128, QBS], F32R, name="pt", tag="pt")
                    nc.scalar.activation(pt[:], st[:], EXP, scale=float(SCALE))
                    i = ki - 4 * qj
                    if i >= 0:          # diagonal tile -> causal mask
                        nc.vector.tensor_mul(pt[:], pt[:],
                                             mask_sb[:, i * QBS:(i + 1) * QBS])
                    nc.tensor.matmul(dn_ps[:], ones_sb[:], pt[:],
                                     start=(ki == 0), stop=(ki == nk - 1))
                    nc.tensor.matmul(o_ps[:],
                                     vh_sb[:, ki * 128:(ki + 1) * 128], pt[:],
                                     start=(ki == 0), stop=(ki == nk - 1))

                inv = bout.tile([1, QBS], F32R, name="inv", tag="inv")
                nc.vector.reciprocal(inv[:], dn_ps[:])
                bc_ps = psdn.tile([128, QBS], F32, name="bc", tag="dn")
                nc.tensor.matmul(bc_ps[:], ones1_sb[:], inv[:],
                                 start=True, stop=True)
                bc_sb = bout.tile([128, QBS], F32, name="bc_sb", tag="bc_sb")
                nc.vector.tensor_copy(bc_sb[:], bc_ps[:])
                onorm = bout.tile([128, QBS], F32R, name="onorm", tag="onorm")
                nc.vector.tensor_mul(onorm[:], o_ps[:], bc_sb[:])
                nc.scalar.dma_start(out=oTd[qj][h * 128:(h + 1) * 128, :], in_=onorm[:])


def _phase_c(nc, tc, wo_sb, oTd, outPT, cps, cot, cout):
    """outPT[d_out, tok] = w_o_slice^T-free partial, feature-major (the host
    transposes). rhs tiles are contiguous row-slices of oTd."""
    if True:
        for tj in range(4):           # token blocks of 512
            ots = []
            for c in range(NLC):
                ot = cot.tile([128, 512], F32R, name="ot", tag="ot")
                nc.sync.dma_start(
                    out=ot[:], in_=oTd[tj][c * 128:(c + 1) * 128, :])
                ots.append(ot)
            for di in range(DM // 128):
                ps = cps.tile([128, 512], F32, name="cpst", tag="cpst")
                for c in range(NLC):
                    nc.tensor.matmul(
                        ps[:],
                        wo_sb[:, c * DM + di * 128: c * DM + (di + 1) * 128],
                        ots[c][:],
                        start=(c == 0), stop=(c == NLC - 1))
                osb = cout.tile([128, 512], F32, name="osb", tag="osb")
                if di % 2 == 0:
                    nc.vector.tensor_copy(osb[:], ps[:])
                else:
                    nc.scalar.copy(osb[:], ps[:])
                nc.scalar.dma_start(out=outPT[di * 128:(di + 1) * 128,
                                               tj * 512:(tj + 1) * 512], in_=osb[:])


def host_prep(x, w_q, w_dkv, w_ukv, w_o, w_q_rot, w_k_rot, mask):
    """Build the 8 per-core input maps (all host-side numpy, fp32)."""
    x = np.asarray(x, np.float32)
    w_q = np.asarray(w_q, np.float32)
    w_dkv = np.asarray(w_dkv, np.float32)
    w_ukv = np.asarray(w_ukv, np.float32)
    w_o = np.asarray(w_o, np.float32)
    w_q_rot = np.asarray(w_q_rot, np.float32)
    w_k_rot = np.asarray(w_k_rot, np.float32)

    inv_freq = (1.0 / (ROPE_BASE ** (np.arange(0, DR, 2, dtype=np.float64) / DR)))
    ang = np.arange(T, dtype=np.float64)[:, None] * inv_freq[None, :]   # [T, 32]
    cosb = np.cos(ang).T.astype(np.float32)     # [32, T]
    sinb = np.sin(ang).T.astype(np.float32)
    cos64 = np.vstack([cosb, cosb])
    ssin64 = np.vstack([-sinb, sinb])
    cos2 = np.ascontiguousarray(np.tile(cos64, (2, 1)))     # [128, T]
    ssin2 = np.ascontiguousarray(np.tile(ssin64, (2, 1)))

    r = np.arange(128)[:, None]
    c = np.arange(512)[None, :]
    dmask = np.concatenate(
        [(128 * i + r <= c).astype(np.float32) for i in range(4)], axis=1)  # [128, 2048]

    wdkvT = np.ascontiguousarray(w_dkv.T)
    wkrT = np.ascontiguousarray(w_k_rot.T)
    wukv4 = w_ukv.reshape(NH, 2, DH, DL)
    ones_in = np.ones((128, 128), np.float32)

    in_maps = []
    for core in range(N_CORES):
        b, g = core // 4, core % 4
        heads = range(4 * g, 4 * g + 4)
        wkT = np.ascontiguousarray(
            np.concatenate([wukv4[h, 0] for h in heads], axis=0).T)  # [DL, GF]
        wvT = np.ascontiguousarray(
            np.concatenate([wukv4[h, 1] for h in heads], axis=0).T)
        in_maps.append({
            "xT": np.ascontiguousarray(x[b].T),
            "wq": np.ascontiguousarray(w_q[g * GF:(g + 1) * GF].T),
            "wdkv": wdkvT,
            "wk": wkT,
            "wv": wvT,
            "wqr": np.ascontiguousarray(w_q_rot[g * QRF:(g + 1) * QRF].T),
            "wkr": wkrT,
            "wo": np.ascontiguousarray(w_o[:, g * GF:(g + 1) * GF].T),
            "cos2": cos2,
            "ssin2": ssin2,
            "dmask": dmask,
            "onesd": ones_in,
        })
    return in_maps


_NC_CACHE = None


def get_nc():
    global _NC_CACHE
    if _NC_CACHE is None:
        _NC_CACHE = build_nc()
    return _NC_CACHE


def kernel(**inputs) -> np.ndarray:
    nc = get_nc()
    in_maps = host_prep(**inputs)
    res = bass_utils.run_bass_kernel_spmd(nc, in_maps, core_ids=list(range(N_CORES)))
    out = np.zeros((B, DM, T), np.float32)
    for core in range(N_CORES):
        out[core // 4] += res.results[core]["outPT"]
    return np.ascontiguousarray(out.transpose(0, 2, 1))



# revision 1
# speedup vs baseline: 2.5040x; 2.5040x over previous
"""MLA-style attention (decoupled-RoPE) Trainium2 Bass kernel, 8-core SPMD.

Sharding: batch (2) x head-group (4 groups of 4 heads) = 8 cores.
Each core computes its batch's tokens for its 4 heads end-to-end
(projections -> rope -> causal softmax attention -> w_o partial),
returning a partial feature-major [D_MODEL, T] output; the host sums the
4 head-group partials per batch element and transposes.

All matmuls run as float32r (tf32-like) on the PE array. Scores are
computed k-major (S^T tiles [tok_k, tok_q]) so softmax denominators come
from a ones-vector matmul and P@V needs no transposes.
"""
import os
import sys

sys.path.insert(0, "/opt/trn_rl_repo")
os.environ.setdefault("JAX_PLATFORMS", "axon")

import numpy as np

import concourse.bacc as bacc
import concourse.mybir as mybir
import concourse.tile as tile
from concourse import bass_utils

# Model constants (hardcoded from the problem spec).
B, T, DM = 2, 2048, 2048
NH, DH, DL, DR = 16, 128, 512, 64
HPC = 4                      # heads per core
GF = HPC * DH                # 512 head-features per core
QRF = HPC * DR               # 256 rope features per core
SCALE = 1.0 / np.sqrt(DH + DR)
ROPE_BASE = 10000.0
N_CORES = 8

F32 = mybir.dt.float32
F32R = mybir.dt.float32r
EXP = mybir.ActivationFunctionType.Exp

TBS = 512                    # token block size in phase A
NTB = T // TBS               # 4
NKC = DM // 128              # 16 contraction chunks over d_model
NLC = DL // 128              # 4 contraction chunks over d_latent
NTC = T // 128               # 16 token chunks


def build_nc(reps=1):
    nc = bacc.Bacc("TRN2", target_bir_lowering=False, debug=False)

    # External inputs (per-core shards, host-prepared)
    xT = nc.dram_tensor("xT", [DM, T], F32R, kind="ExternalInput").ap()
    wq = nc.dram_tensor("wq", [DM, GF], F32R, kind="ExternalInput").ap()
    wdkv = nc.dram_tensor("wdkv", [DM, DL], F32R, kind="ExternalInput").ap()
    wk = nc.dram_tensor("wk", [DL, GF], F32R, kind="ExternalInput").ap()
    wv = nc.dram_tensor("wv", [DL, GF], F32R, kind="ExternalInput").ap()
    wqr = nc.dram_tensor("wqr", [DM, QRF], F32R, kind="ExternalInput").ap()
    wkr = nc.dram_tensor("wkr", [DM, DR], F32R, kind="ExternalInput").ap()
    wo = nc.dram_tensor("wo", [GF, DM], F32R, kind="ExternalInput").ap()
    cos2 = nc.dram_tensor("cos2", [128, T], F32, kind="ExternalInput").ap()
    ssin2 = nc.dram_tensor("ssin2", [128, T], F32, kind="ExternalInput").ap()
    dmask = nc.dram_tensor("dmask", [128, 2048], F32, kind="ExternalInput").ap()
    onesd = nc.dram_tensor("onesd", [128, 128], F32R, kind="ExternalInput").ap()

    outPT = nc.dram_tensor("outPT", [DM, T], F32, kind="ExternalOutput").ap()

    with tile.TileContext(nc) as tc, \
         nc.allow_low_precision(reason="float32r matmul operands are intentional"):
        with tc.tile_pool(name="dstage", bufs=1, space="DRAM") as dram, \
             tc.tile_pool(name="gfix", bufs=1) as gfix:
            # per-token-block staging tiles -> fine-grained cross-phase deps
            qTd = [dram.tile([GF, TBS], F32R, name=f"qTd{i}") for i in range(NTB)]
            qrTd = [dram.tile([QRF, TBS], F32R, name=f"qrTd{i}") for i in range(NTB)]
            oTd = [dram.tile([GF, 512], F32R, name=f"oTd{i}") for i in range(NTB)]

            mask_sb = gfix.tile([128, 2048], F32, name="mask_sb")
            ones_sb = gfix.tile([128, 1], F32R, name="ones_sb")
            ones1_sb = gfix.tile([1, 128], F32R, name="ones1_sb")

            from contextlib import ExitStack
            for _rep in range(reps):
                # k/v/k_rope stay resident in SBUF from projection to attention
                with tc.tile_pool(name="kv", bufs=1) as kvp:
                    kcs = [kvp.tile([128, T], F32R, name=f"kc_sb{h}")
                           for h in range(HPC)]
                    vhs = [kvp.tile([128, T], F32R, name=f"vh_sb{h}")
                           for h in range(HPC)]
                    krope_sb = kvp.tile([64, T], F32R, name="krope_sb")

                    bq_stack = ExitStack()
                    bq = bq_stack.enter_context(tc.tile_pool(name="bq", bufs=2))
                    # A2's weights: bulk SWDGE prefetch that overlaps A1
                    aw2_stack = ExitStack()
                    aw2 = aw2_stack.enter_context(tc.tile_pool(name="aw2", bufs=1))
                    wq_sb = aw2.tile([128, NKC * GF], F32R, name="wq_sb")
                    wqr_sb = aw2.tile([128, NKC * QRF], F32R, name="wqr_sb")
                    wkr_sb = aw2.tile([128, NKC * DR], F32R, name="wkr_sb")

                    def emit_prefetch():
                        # A2 weights ride SWDGE mid-A1 (off the critical
                        # start-up stream, done well before A2 needs them)
                        nc.gpsimd.dma_start(
                            out=wq_sb.rearrange("p (c f) -> p c f", c=NKC),
                            in_=wq.rearrange("(c p) f -> p c f", p=128))
                        nc.gpsimd.dma_start(
                            out=wqr_sb.rearrange("p (c f) -> p c f", c=NKC),
                            in_=wqr.rearrange("(c p) f -> p c f", p=128))
                        nc.gpsimd.dma_start(
                            out=wkr_sb.rearrange("p (c f) -> p c f", c=NKC),
                            in_=wkr.rearrange("(c p) f -> p c f", p=128))

                    _phase_a1(nc, tc, xT, wdkv, wk, wv, kcs, vhs, emit_prefetch)
                    # attention constants: loaded while A2 runs
                    nc.gpsimd.dma_start(out=mask_sb[:], in_=dmask)
                    nc.gpsimd.dma_start(out=ones_sb[:], in_=onesd[:, 0:1])
                    nc.gpsimd.dma_start(out=ones1_sb[:], in_=onesd[0:1, :])
                    _phase_a2(nc, tc, xT, wq_sb, wqr_sb, wkr_sb, cos2, ssin2,
                              qTd, qrTd, krope_sb)
                    aw2_stack.close()
                    with (
                        tc.tile_pool(name="cw", bufs=1) as cw,
                        tc.tile_pool(name="cps", bufs=2, space="PSUM") as cps,
                        tc.tile_pool(name="cot", bufs=8) as cot,
                        tc.tile_pool(name="cout", bufs=6) as cout,
                    ):
                        wo_sb = cw.tile([128, NLC * DM], F32R, name="wo_sb")

                        def emit_wo_chunk(c):
                            nc.gpsimd.dma_start(
                                out=wo_sb[:, c * DM:(c + 1) * DM],
                                in_=wo[c * 128:(c + 1) * 128, :])

                        _phase_b(nc, tc, mask_sb, ones_sb, ones1_sb, emit_wo_chunk, bq,
                                 qTd, qrTd, kcs, vhs, krope_sb, oTd)
                        _phase_c(nc, tc, wo_sb, oTd, outPT, cps, cot, cout)
                    bq_stack.close()

    nc.compile()
    return nc


def _phase_a1(nc, tc, xT, wdkv, wk, wv, kcs, vhs, emit_prefetch):
    """dkv projection -> k (feature-major) and v (token-major), written
    directly into the resident SBUF tiles used by attention."""
    with (
        tc.tile_pool(name="aw1", bufs=1) as aw,
        tc.tile_pool(name="ax1", bufs=4) as ax,
        tc.tile_pool(name="adkv", bufs=1) as adkv,
        tc.tile_pool(name="aps1", bufs=8, space="PSUM") as aps,
    ):
        wdkv_sb = aw.tile([128, NKC * DL], F32R, name="wdkv_sb")
        xts0 = []
        for c in range(NKC):
            nc.sync.dma_start(out=wdkv_sb[:, c * DL:(c + 1) * DL],
                              in_=wdkv[c * 128:(c + 1) * 128, :])
            xt = ax.tile([128, TBS], F32R, name="xt", tag="xt")
            nc.sync.dma_start(out=xt[:], in_=xT[c * 128:(c + 1) * 128, 0:TBS])
            xts0.append(xt)
        wk_sb = aw.tile([128, NLC * GF], F32R, name="wk_sb")
        wv_sb = aw.tile([128, NLC * GF], F32R, name="wv_sb")
        for c in range(NLC):
            nc.sync.dma_start(out=wk_sb[:, c * GF:(c + 1) * GF],
                              in_=wk[c * 128:(c + 1) * 128, :])
            nc.sync.dma_start(out=wv_sb[:, c * GF:(c + 1) * GF],
                              in_=wv[c * 128:(c + 1) * 128, :])

        for tb in range(NTB):
            if tb == 1:
                emit_prefetch()
            tsl = slice(tb * TBS, (tb + 1) * TBS)
            dkv_ps = [aps.tile([128, TBS], F32, name="ps", tag="ps") for _ in range(4)]
            for ki in range(NKC):
                if tb == 0:
                    xt = xts0[ki]
                else:
                    xt = ax.tile([128, TBS], F32R, name="xt", tag="xt")
                    nc.sync.dma_start(out=xt[:], in_=xT[ki * 128:(ki + 1) * 128, tsl])
                for fi in range(4):
                    nc.tensor.matmul(
                        dkv_ps[fi][:],
                        wdkv_sb[:, ki * DL + fi * 128: ki * DL + (fi + 1) * 128],
                        xt[:],
                        start=(ki == 0), stop=(ki == NKC - 1))
            dkv_sb = adkv.tile([128, NLC * TBS], F32R, name="dkv_sb", tag="dkv")
            for fi in range(4):
                dst = dkv_sb[:, fi * TBS:(fi + 1) * TBS]
                if fi % 2 == 0:
                    nc.vector.tensor_copy(dst, dkv_ps[fi][:])
                else:
                    nc.scalar.copy(dst, dkv_ps[fi][:])

            for fi in range(4):   # k feature tiles (one head each)
                kps = aps.tile([128, TBS], F32, name="ps", tag="ps")
                for c in range(NLC):
                    nc.tensor.matmul(
                        kps[:],
                        wk_sb[:, c * GF + fi * 128: c * GF + (fi + 1) * 128],
                        dkv_sb[:, c * TBS:(c + 1) * TBS],
                        start=(c == 0), stop=(c == NLC - 1))
                nc.scalar.copy(kcs[fi][:, tsl], kps[:])
            for ts in range(4):   # v token tiles (token-major)
                vps = aps.tile([128, TBS], F32, name="ps", tag="ps")
                for c in range(NLC):
                    nc.tensor.matmul(
                        vps[:],
                        dkv_sb[:, c * TBS + ts * 128: c * TBS + (ts + 1) * 128],
                        wv_sb[:, c * GF:(c + 1) * GF],
                        start=(c == 0), stop=(c == NLC - 1))
                cc = tb * 4 + ts
                for h in range(HPC):
                    nc.vector.tensor_copy(vhs[h][:, cc * 128:(cc + 1) * 128],
                                          vps[:, h * 128:(h + 1) * 128])


def _phase_a2(nc, tc, xT, wq_sb, wqr_sb, wkr_sb, cos2, ssin2,
              qTd, qrTd, krope_sb):
    """q / q_rot / k_rot projections + rope; q staged to DRAM per block,
    k_rope written into its resident SBUF tile."""
    with (
        tc.tile_pool(name="ax2", bufs=6) as ax,
        tc.tile_pool(name="astage", bufs=4) as ast,
        tc.tile_pool(name="arope", bufs=2) as arp,
        tc.tile_pool(name="aps2", bufs=8, space="PSUM") as aps,
    ):
        xts0 = []
        for c in range(NKC):
            xt = ax.tile([128, TBS], F32R, name="xt2", tag="xt2")
            nc.sync.dma_start(out=xt[:], in_=xT[c * 128:(c + 1) * 128, 0:TBS])
            xts0.append(xt)

        for tb in range(NTB):
            tsl = slice(tb * TBS, (tb + 1) * TBS)
            q_ps = [aps.tile([128, TBS], F32, name="ps2", tag="ps2") for _ in range(4)]
            qr_ps = [aps.tile([128, TBS], F32, name="ps2", tag="ps2") for _ in range(2)]
            kr_ps = aps.tile([64, TBS], F32, name="ps2", tag="ps2")
            cos_sl = arp.tile([128, TBS], F32, name="cos_sl", tag="cos_sl")
            nc.sync.dma_start(out=cos_sl[:], in_=cos2[:, tsl])
            ssin_sl = arp.tile([128, TBS], F32, name="ssin_sl", tag="ssin_sl")
            nc.sync.dma_start(out=ssin_sl[:], in_=ssin2[:, tsl])
            for ki in range(NKC):
                if tb == 0:
                    xt = xts0[ki]
                else:
                    xt = ax.tile([128, TBS], F32R, name="xt2", tag="xt2")
                    nc.sync.dma_start(out=xt[:], in_=xT[ki * 128:(ki + 1) * 128, tsl])
                for fi in range(4):
                    nc.tensor.matmul(
                        q_ps[fi][:],
                        wq_sb[:, ki * GF + fi * 128: ki * GF + (fi + 1) * 128],
                        xt[:], start=(ki == 0), stop=(ki == NKC - 1))
                for fi in range(2):
                    nc.tensor.matmul(
                        qr_ps[fi][:],
                        wqr_sb[:, ki * QRF + fi * 128: ki * QRF + (fi + 1) * 128],
                        xt[:], start=(ki == 0), stop=(ki == NKC - 1))
                nc.tensor.matmul(
                    kr_ps[:], wkr_sb[:, ki * DR:(ki + 1) * DR], xt[:],
                    start=(ki == 0), stop=(ki == NKC - 1))
            for fi in range(4):
                qst = ast.tile([128, TBS], F32R, name="st", tag="st")
                nc.vector.tensor_copy(qst[:], q_ps[fi][:])
                nc.scalar.dma_start(out=qTd[tb][fi * 128:(fi + 1) * 128, :], in_=qst[:])

            # rope on q_rot tiles (2 tiles of 2 heads) and k_rot
            for fi in range(2):
                raw = arp.tile([128, TBS], F32, name="rraw", tag="rraw")
                nc.scalar.copy(raw[:], qr_ps[fi][:])
                sh = arp.tile([128, TBS], F32, name="rsh", tag="rsh")
                for bb in range(4):
                    sb_ = bb ^ 1  # swap 32-halves within each 64-block
                    nc.gpsimd.dma_start(out=sh[bb * 32:(bb + 1) * 32, :],
                                        in_=raw[sb_ * 32:(sb_ + 1) * 32, :])
                m1 = arp.tile([128, TBS], F32, name="rm1", tag="rm1")
                nc.vector.tensor_mul(m1[:], raw[:], cos_sl[:])
                m2 = arp.tile([128, TBS], F32, name="rm2", tag="rm2")
                nc.vector.tensor_mul(m2[:], sh[:], ssin_sl[:])
                ro = ast.tile([128, TBS], F32R, name="st", tag="st")
                nc.vector.tensor_add(ro[:], m1[:], m2[:])
                nc.scalar.dma_start(out=qrTd[tb][fi * 128:(fi + 1) * 128, :], in_=ro[:])

            kraw = arp.tile([64, TBS], F32, name="rraw", tag="rraw")
            nc.scalar.copy(kraw[:], kr_ps[:])
            ksh = arp.tile([64, TBS], F32, name="rsh", tag="rsh")
            nc.gpsimd.dma_start(out=ksh[0:32, :], in_=kraw[32:64, :])
            nc.gpsimd.dma_start(out=ksh[32:64, :], in_=kraw[0:32, :])
            km1 = arp.tile([64, TBS], F32, name="rm1", tag="rm1")
            nc.vector.tensor_mul(km1[:], kraw[:], cos_sl[0:64, :])
            km2 = arp.tile([64, TBS], F32, name="rm2", tag="rm2")
            nc.vector.tensor_mul(km2[:], ksh[:], ssin_sl[0:64, :])
            nc.vector.tensor_add(krope_sb[:, tsl], km1[:], km2[:])


def _phase_b(nc, tc, mask_sb, ones_sb, ones1_sb, emit_wo_chunk, bq, qTd, qrTd, kcs, vhs,
             krope_sb, oTd):
    """Causal attention: q-block outer, head inner (so oTd completes
    front-to-back and phase C can overlap). k/v/k_rope already resident."""
    QBS = 512                 # q block size
    NQB = T // QBS            # 4
    with (
        tc.tile_pool(name="bpt", bufs=3) as bpt,
        tc.tile_pool(name="bout", bufs=3) as bout,
        tc.tile_pool(name="psst", bufs=3, space="PSUM") as psst,
        tc.tile_pool(name="pso", bufs=2, space="PSUM") as pso,
        tc.tile_pool(name="psdn", bufs=1, space="PSUM") as psdn,
    ):
        for qj in range(NQB):
            emit_wo_chunk(qj)
            qsl = slice(qj * QBS, (qj + 1) * QBS)
            nk = 4 * (qj + 1)         # causal: tok_k tiles 0..nk-1
            for h in range(HPC):
                kc_sb, vh_sb = kcs[h], vhs[h]
                qc = bq.tile([128, QBS], F32R, name="qc", tag="qc")
                nc.sync.dma_start(out=qc[:], in_=qTd[qj][h * 128:(h + 1) * 128, :])
                qr = bq.tile([64, QBS], F32R, name="qr", tag="qr")
                nc.sync.dma_start(out=qr[:], in_=qrTd[qj][h * 64:(h + 1) * 64, :])

                dn_ps = psdn.tile([1, QBS], F32, name="dn", tag="dn")
                o_ps = pso.tile([128, QBS], F32, name="o", tag="o")

                def emit_st(ki):
                    st = psst.tile([128, QBS], F32, name="stp", tag="stp")
                    nc.tensor.matmul(
                        st[:], kc_sb[:, ki * 128:(ki + 1) * 128], qc[:],
                        start=True, stop=False)
                    nc.tensor.matmul(
                        st[:], krope_sb[:, ki * 128:(ki + 1) * 128], qr[:],
                        start=False, stop=True)
                    return st

                sts = {0: emit_st(0), 1: emit_st(1)}
                for ki in range(nk):
                    if ki + 2 < nk:
                        sts[ki + 2] = emit_st(ki + 2)
                    st = sts.pop(ki)
                    pt = bpt.tile([128, QBS], F32R, name="pt", tag="pt")
                    nc.scalar.activation(pt[:], st[:], EXP, scale=float(SCALE))
                    i = ki - 4 * qj
                    if i >= 0:          # diagonal tile -> causal mask
                        nc.vector.tensor_mul(pt[:], pt[:],
                                             mask_sb[:, i * QBS:(i + 1) * QBS])
                    nc.tensor.matmul(dn_ps[:], ones_sb[:], pt[:],
                                     start=(ki == 0), stop=(ki == nk - 1))
                    nc.tensor.matmul(o_ps[:],
                                     vh_sb[:, ki * 128:(ki + 1) * 128], pt[:],
                                     start=(ki == 0), stop=(ki == nk - 1))

                inv = bout.tile([1, QBS], F32R, name="inv", tag="inv")
                nc.vector.reciprocal(inv[:], dn_ps[:])
                bc_ps = psdn.tile([128, QBS], F32, name="bc", tag="dn")
                nc.tensor.matmul(bc_ps[:], ones1_sb[:], inv[:],
                                 start=True, stop=True)
                bc_sb = bout.tile([128, QBS], F32, name="bc_sb", tag="bc_sb")
                nc.vector.tensor_copy(bc_sb[:], bc_ps[:])
                onorm = bout.tile([128, QBS], F32R, name="onorm", tag="onorm")
                nc.vector.tensor_mul(onorm[:], o_ps[:], bc_sb[:])
                nc.scalar.dma_start(out=oTd[qj][h * 128:(h + 1) * 128, :], in_=onorm[:])


def _phase_c(nc, tc, wo_sb, oTd, outPT, cps, cot, cout):
    """outPT[d_out, tok] = w_o_slice^T-free partial, feature-major (the host
    transposes). rhs tiles are contiguous row-slices of oTd."""
    if True:
        for tj in range(4):           # token blocks of 512
            ots = []
            for c in range(NLC):
                ot = cot.tile([128, 512], F32R, name="ot", tag="ot")
                nc.sync.dma_start(
                    out=ot[:], in_=oTd[tj][c * 128:(c + 1) * 128, :])
                ots.append(ot)
            for di in range(DM // 128):
                ps = cps.tile([128, 512], F32, name="cpst", tag="cpst")
                for c in range(NLC):
                    nc.tensor.matmul(
                        ps[:],
                        wo_sb[:, c * DM + di * 128: c * DM + (di + 1) * 128],
                        ots[c][:],
                        start=(c == 0), stop=(c == NLC - 1))
                osb = cout.tile([128, 512], F32, name="osb", tag="osb")
                if di % 2 == 0:
                    nc.vector.tensor_copy(osb[:], ps[:])
                else:
                    nc.scalar.copy(osb[:], ps[:])
                nc.scalar.dma_start(out=outPT[di * 128:(di + 1) * 128,
                                               tj * 512:(tj + 1) * 512], in_=osb[:])


def host_prep(x, w_q, w_dkv, w_ukv, w_o, w_q_rot, w_k_rot, mask):
    """Build the 8 per-core input maps (all host-side numpy, fp32)."""
    x = np.asarray(x, np.float32)
    w_q = np.asarray(w_q, np.float32)
    w_dkv = np.asarray(w_dkv, np.float32)
    w_ukv = np.asarray(w_ukv, np.float32)
    w_o = np.asarray(w_o, np.float32)
    w_q_rot = np.asarray(w_q_rot, np.float32)
    w_k_rot = np.asarray(w_k_rot, np.float32)

    inv_freq = (1.0 / (ROPE_BASE ** (np.arange(0, DR, 2, dtype=np.float64) / DR)))
    ang = np.arange(T, dtype=np.float64)[:, None] * inv_freq[None, :]   # [T, 32]
    cosb = np.cos(ang).T.astype(np.float32)     # [32, T]
    sinb = np.sin(ang).T.astype(np.float32)
    cos64 = np.vstack([cosb, cosb])
    ssin64 = np.vstack([-sinb, sinb])
    cos2 = np.ascontiguousarray(np.tile(cos64, (2, 1)))     # [128, T]
    ssin2 = np.ascontiguousarray(np.tile(ssin64, (2, 1)))

    r = np.arange(128)[:, None]
    c = np.arange(512)[None, :]
    dmask = np.concatenate(
        [(128 * i + r <= c).astype(np.float32) for i in range(4)], axis=1)  # [128, 2048]

    wdkvT = np.ascontiguousarray(w_dkv.T)
    wkrT = np.ascontiguousarray(w_k_rot.T)
    wukv4 = w_ukv.reshape(NH, 2, DH, DL)
    ones_in = np.ones((128, 128), np.float32)

    in_maps = []
    for core in range(N_CORES):
        b, g = core // 4, core % 4
        heads = range(4 * g, 4 * g + 4)
        wkT = np.ascontiguousarray(
            np.concatenate([wukv4[h, 0] for h in heads], axis=0).T)  # [DL, GF]
        wvT = np.ascontiguousarray(
            np.concatenate([wukv4[h, 1] for h in heads], axis=0).T)
        in_maps.append({
            "xT": np.ascontiguousarray(x[b].T),
            "wq": np.ascontiguousarray(w_q[g * GF:(g + 1) * GF].T),
            "wdkv": wdkvT,
            "wk": wkT,
            "wv": wvT,
            "wqr": np.ascontiguousarray(w_q_rot[g * QRF:(g + 1) * QRF].T),
            "wkr": wkrT,
            "wo": np.ascontiguousarray(w_o[:, g * GF:(g + 1) * GF].T),
            "cos2": cos2,
            "ssin2": ssin2,
            "dmask": dmask,
            "onesd": ones_in,
        })
    return in_maps


_NC_CACHE = None


def get_nc():
    global _NC_CACHE
    if _NC_CACHE is None:
        _NC_CACHE = build_nc()
    return _NC_CACHE


def kernel(**inputs) -> np.ndarray:
    nc = get_nc()
    in_maps = host_prep(**inputs)
    res = bass_utils.run_bass_kernel_spmd(nc, in_maps, core_ids=list(range(N_CORES)))
    out = np.zeros((B, DM, T), np.float32)
    for core in range(N_CORES):
        out[core // 4] += res.results[core]["outPT"]
    return np.ascontiguousarray(out.transpose(0, 2, 1))

